# revision 7
# baseline (speedup 1.0000x reference)
"""OccupancyToTopology Trainium2 kernel (bf16-output 2x-mode pipeline).

Input: occupancy [65, 65, 65] f32 on a (W+1,H+1,D+1) grid, W=H=D=64.
Output: topo [262144, 256] f32 where topo[n, t] = prod_c (p_c if bit_c(t) else 1-p_c),
with n = x*4096 + y*64 + z and the 8 cell corners in marching-cubes order
  CORNER_OFFSETS = [(0,0,0),(1,0,0),(1,1,0),(0,1,0),(0,0,1),(1,0,1),(1,1,1),(0,1,1)]
(offsets are (dx,dy,dz); bit c of t selects corner c).

Sharding: x split across 8 cores; core k owns cells x in [8k, 8k+8) and gets the
occupancy slab occupancy[8k:8k+9] (1-plane halo). Output rows are fully local.

Per-core pipeline (partitions p = x2*64 + y for a group of two x-planes):
  Corners pair up as (c, c+4) sharing (dx,dy): the oz in {0,1} halves are
  z-shifted views of one gathered row, so 4 row-gather DMAs per group cover all
  8 corners. All 16 gathers are issued up front; trigger instructions cost
  ~0.5-1us each on their engine, so they are spread over the sync and GpSimd
  queues and none go on ScalarE (which must start terms immediately):
    RAB rows: rh=0 (pair-LO): rr=0 -> (dx,dy)=(0,0) [c0/c4], rr=1 -> (1,1) [c2/c6]
              rh=1 (pair-HI): rr=0 -> (1,0) [c1/c5],          rr=1 -> (0,1) [c3/c7]
  Slot s = rr*2+oz orders pairs as [pair0, pair2, pair1, pair3]: quad-lo
  operands are slots {0,1}, quad-hi slots {2,3}.

    terms (ScalarE, 4 ops): T8 (rh2, s4, z64, b2) f32, b=0 half 1-p, b=1 half p
    pairs (1 TT, f32 1x):   P4ALL (s4, z64, bh2, bl2) f32        [FD 1024]
    quads (2 TT, f32->bf16): Q16ALL (lh2, z64, jh4, jl4) bf16    [FD 1024 x2]
       lh=0 is L16 (z, l) = t bits 0-3; lh=1 is H16 (z, h) = bits 4-7;
       H runs first so its dup overlaps the L quad
    dup (2 ops, z-halved): H16 -> H16D (z64, h16, d2) bf16
      (ScalarE for steady groups; DVE tensor_copy for group 0 where the
      list scheduler otherwise parks the dup behind stalled ops)
    combine (TT bf16 @2x):  per z-chunk OUT[z,h,l] = L16[z,l]*H16D[z,h]
      Emission is software-pipelined: group g+1's pair/quads/dups are
      emitted BETWEEN group g's combine chunks, so DVE output production
      never pauses more than one stage op and the HBM store drain never
      starves. Group 0 leads with a z-quarter combine (earliest first
      store); the last group ends with z-quarter chunks (smallest exposed
      final drain).

  The combine runs in DVE 2x_1p mode (2 elem/cycle, hardware-verified): it
  needs EVERY operand AP innermost [stride +-1, count>=2] and 16-bit dtype,
  which is why H16 is materialized duplicated x2 (H16D). Compiler limit:
  every operand must optimize to <=3 free AP dims, which constrains which
  stages can be z-sliced (pair/quad merges only hold at full z).

  Stores are bf16 (halves the f32 store traffic; measured store bandwidth
  ~490 GB/s/core makes DVE, not DMA, the critical path). The host converts
  back to f32. Error: only the two quad outputs and the combine output are
  rounded to bf16 -> 3 truncation units ~ 1.07e-2 max rel err measured,
  inside the 2e-2 gate (an all-bf16 tree accumulates 15 units ~ 3.4e-2:
  measured, fails).

  Measured: ~76.1-81us on 8 cores (was 152.5us f32/1x baseline). DVE is
  saturated with zero mid-stream gaps: ~48.6us busy at 0.96GHz (some runs
  clock at 0.8GHz -> ~58.6us, hence the spread), plus ~15us head (7.5us
  NEFF preamble + gather + terms) and a store-drain tail (stores are
  HBM-limited ~360-430GB/s/core; the last group's output drains ~10us past
  the final combine). Combine z-quarter chunks + stores alternating over
  the sync and GpSimd queues keep the drain as early as possible; ScalarE
  must NOT carry store triggers (they head-of-line block the dup ops).
"""

import sys

if "/opt/trn_rl_repo" not in sys.path:
    sys.path.insert(0, "/opt/trn_rl_repo")

import numpy as np

import concourse.bass as bass
import concourse.mybir as mybir
from concourse.bass_utils import run_bass_kernel_spmd
from concourse.tile import TileContext

F32 = mybir.dt.float32
BF16 = mybir.dt.bfloat16
N_CORES = 8
W = H = D = 64
XPC = W // N_CORES          # x-planes of cells per core = 8
N_LOCAL = XPC * H * D       # cells per core = 32768
N_GROUPS = XPC // 2         # two x-planes of cells per group = 4


def _hoist_extra_waits(nc):
    """Walrus on this toolchain rejects instructions carrying more than one
    sync-wait. Hoist every wait of a multi-wait instruction into standalone
    EventSemaphore instructions just before it in the same engine stream."""
    ctr = 0
    for fn in nc.m.functions:
        for blk in fn.blocks:
            new_insts = []
            for inst in blk.instructions:
                si = inst.sync_info
                waits = list(si.on_wait) if (si is not None and si.on_wait) else []
                if len(waits) > 1:
                    # DMA-vs-DMA ordering guards (DMAHW/DMASW lane sems) stay
                    # on the DMA itself; everything else becomes a standalone
                    # sequencer wait right before it.
                    keep = []
                    if inst.opcode in ("DMACopy", "TensorLoad", "TensorSave"):
                        for w in waits:
                            if "DMAHW" in w.ant_name or "DMASW" in w.ant_name:
                                keep = [w]
                                break
                    if not keep:
                        keep = [waits[-1]]
                    hoisted = [w for w in waits if w is not keep[0]]
                    for w in hoisted:
                        ev = mybir.InstEventSemaphore(
                            name=f"hoistw-{ctr}", ins=[], outs=[])
                        ctr += 1
                        ev.engine = inst.engine
                        ev.sync_info = mybir.SyncInfo(on_wait=[w], on_update=[])
                        new_insts.append(ev)
                    inst.sync_info = mybir.SyncInfo(
                        on_wait=keep, on_update=list(si.on_update))
                new_insts.append(inst)
            blk.instructions = new_insts


def _build_program(hoist=True):
    nc = bass.Bass()
    occ = nc.dram_tensor("occ", [XPC + 1, H + 1, D + 1], F32, kind="ExternalInput")
    topo = nc.dram_tensor("topo", [N_LOCAL, 256], BF16, kind="ExternalOutput")
    topo_ap = topo[:, :]
    Copy = mybir.ActivationFunctionType.Copy

    with TileContext(nc) as tc:
        with (
            tc.tile_pool(name="raw", bufs=1) as raw_pool,
            tc.tile_pool(name="term", bufs=2) as term_pool,
            tc.tile_pool(name="stage", bufs=2) as stage_pool,
            tc.tile_pool(name="out", bufs=2) as out_pool,
        ):
            # ---- all row gathers up front. Trigger instructions cost
            # ~0.5-1us EACH on their engine, so none go on ScalarE (it must
            # start terms immediately): group 0 rides the sync queue (free
            # until the first store), groups 1-3 ride GpSimd.
            rabs = []
            for g in range(N_GROUPS):
                x0 = g * 2
                rab = raw_pool.tile([128, 4 * 65], F32, tag=f"rab{g}")
                rv = rab.rearrange("p (rh rr z) -> p rh rr z", rh=2, rr=2)
                if g == 0:
                    engs = (nc.sync, nc.gpsimd, nc.sync, nc.scalar)
                elif g == 1:
                    engs = (nc.gpsimd,) * 4
                else:
                    engs = (nc.sync,) * 4
                engs[0].dma_start(out=rv[:, 0:1, 0:1], in_=occ[x0:x0 + 2, 0:64, :])
                engs[1].dma_start(out=rv[:, 0:1, 1:2], in_=occ[x0 + 1:x0 + 3, 1:65, :])
                engs[2].dma_start(out=rv[:, 1:2, 0:1], in_=occ[x0 + 1:x0 + 3, 0:64, :])
                engs[3].dma_start(out=rv[:, 1:2, 1:2], in_=occ[x0:x0 + 2, 1:65, :])
                rabs.append(rab)

            # warm the ScalarE activation table while gathers are in flight
            # (the first ACTIVATE otherwise pays a ~1.3us ACT_TABLE_LOAD on
            # the critical path).
            warm = raw_pool.tile([128, 2], F32, tag="warm")
            nc.vector.memset(warm[:, 0:1], 0.0)
            nc.scalar.activation(warm[:, 1:2], warm[:, 0:1], Copy)

            def emit_terms(g):
                """terms (f32): b=0 half is 1-p, b=1 half is p; the oz
                z-window shift makes slot s = (rr, oz) of the 4 gathered
                rows. For group 0 the p-halves go to DVE (idle during the
                head) so the serial terms wall before the first pair halves.
                """
                rab_v = rabs[g].rearrange("p (rh rr z) -> p rh rr z",
                                          rh=2, rr=2)
                t8 = term_pool.tile([128, 2 * 4 * 64 * 2], F32, tag="t8")
                t8_v = t8.rearrange("p (rh rr oz z b) -> p rh rr oz z b",
                                    rh=2, rr=2, oz=2, z=64, b=2)
                for oz in (0, 1):
                    src = rab_v[:, :, :, None, oz:oz + 64, None]
                    nc.scalar.activation(t8_v[:, :, :, oz:oz + 1, :, 0:1], src,
                                         Copy, bias=1.0, scale=-1.0)
                    if g == 0:
                        nc.vector.tensor_copy(
                            t8_v[:, :, :, oz:oz + 1, :, 1:2], src)
                    else:
                        nc.scalar.activation(t8_v[:, :, :, oz:oz + 1, :, 1:2],
                                             src, Copy)
                return t8

            class Stage:
                pass

            def emit_pair(g, t8):
                """pairs: ONE TT op, f32.
                P4ALL[s, z, bh, bl] = T8[lo, s, z, bl] * T8[hi, s, z, bh]"""
                st = Stage()
                st.p4all = stage_pool.tile([128, 4 * 64 * 4], F32, tag="p4all")
                p4_v = st.p4all.rearrange("p (s z bh bl) -> p s z bh bl",
                                          s=4, z=64, bh=2, bl=2)[:, None]
                t8_s = t8.rearrange("p (rh s z b) -> p rh s z b",
                                    rh=2, s=4, z=64, b=2)
                lo_v = t8_s[:, 0:1, :, :, None, :] \
                    .broadcast_to([128, 1, 4, 64, 2, 2])
                hi_v = t8_s[:, 1:2, :, :, :, None] \
                    .broadcast_to([128, 1, 4, 64, 2, 2])
                nc.vector.tensor_mul(p4_v, lo_v, hi_v)
                st.q16all = stage_pool.tile([128, 2 * 64 * 16], BF16,
                                            tag="q16all")
                st.h16d = stage_pool.tile([128, 64 * 16 * 2], BF16, tag="h16d")
                return st

            def emit_quad(g, st, lh):
                """quads: per-lh TT, f32 in -> bf16 out, H (lh=1) first so
                its dup can run while the L quad executes.
                Q16ALL[lh, z, jh, jl] = P4ALL[lh, z, jl] * P4ALL[2+lh, z, jh]
                """
                q16_v = st.q16all.rearrange("p (lh z jh jl) -> p lh z jh jl",
                                            lh=2, z=64, jh=4, jl=4)
                p4_s = st.p4all.rearrange("p (s z j) -> p s z j",
                                          s=4, z=64, j=4)
                ql_v = p4_s[:, lh:lh + 1, :, None, :] \
                    .broadcast_to([128, 1, 64, 4, 4])
                qh_v = p4_s[:, 2 + lh:3 + lh, :, :, None] \
                    .broadcast_to([128, 1, 64, 4, 4])
                nc.vector.tensor_mul(q16_v[:, lh:lh + 1], ql_v, qh_v)

            def emit_dup(g, st, z0, scalar=None):
                """combine-hi dup: H16 (lh=1) -> (z, h16, d2) bf16 on ScalarE
                (group 0's are emitted before terms(1) so they are not parked
                behind its gather wait)."""
                h16d_dst = st.h16d.rearrange("p (z h d) -> p z h d",
                                             z=64, h=16, d=2)
                q16_zh = st.q16all.rearrange("p (lh z h) -> p lh z h",
                                             lh=2, z=64, h=16)
                h16_src = q16_zh[:, 1:2, z0:z0 + 32, :, None] \
                    .broadcast_to([128, 1, 32, 16, 2])
                nc.scalar.activation(h16d_dst[:, None, z0:z0 + 32],
                                     h16_src, Copy)

            def emit_comb(g, st, c0, cn, ci):
                """final combine @2x + store for one z-chunk.
                OUT[z, h, l] = L16[z, l] * H16D[z, h, .]"""
                if ci == 0:
                    st.out_t = out_pool.tile([128, D * 256], BF16, tag="topo")
                out_zv = st.out_t.rearrange("p (z h l8 l2) -> p z h l8 l2",
                                            z=D, h=16, l8=8, l2=2)
                q16_zl = st.q16all.rearrange("p (lh z l8 l2) -> p lh z l8 l2",
                                             lh=2, z=64, l8=8, l2=2)
                h16d_v = st.h16d.rearrange("p (z h d) -> p z h d",
                                           z=64, h=16, d=2)
                l_v = q16_zl[:, 0, c0:c0 + cn][:, :, None, :, :] \
                    .broadcast_to([128, cn, 16, 8, 2])
                h_v = h16d_v[:, c0:c0 + cn][:, :, :, None, :] \
                    .broadcast_to([128, cn, 16, 8, 2])
                nc.vector.tensor_mul(out_zv[:, c0:c0 + cn], l_v, h_v)
                # store rows (x2, y, c0..c0+cn): per partition cn/2 KiB
                # contiguous in HBM at (x2*4096 + y*64 + c0)*256 elements.
                # Each chunk's store is SPLIT across the Sync and Pool DMA
                # queues so both drain concurrently (single-queue plateaus at
                # ~260 GB/s; two busy queues reach ~390-500 GB/s). Never
                # ScalarE: triggers there head-of-line block the dup ops.
                hn = cn // 2
                for s0, sn, st_eng in ((c0, hn, nc.sync),
                                       (c0 + hn, cn - hn, nc.gpsimd)):
                    dst = bass.AP(
                        tensor=topo_ap.tensor,
                        offset=topo_ap.offset + (g * 2 * H * D + s0) * 256,
                        ap=[[4096 * 256, 2], [D * 256, H], [1, sn * 256]],
                    )
                    st_eng.dma_start(
                        out=dst,
                        in_=st.out_t[:, s0 * 256:(s0 + sn) * 256],
                    )

            # Software-pipelined emission: each group's stage ops (pair,
            # quads, dups) are emitted BETWEEN the previous group's combine
            # quarters, so DVE output production never pauses longer than
            # one stage op and the HBM store drain never starves.
            # half chunks (quarters cost ~2us of extra op/trigger overhead,
            # measured); group 0 leads with a z-quarter so the first store
            # fires earliest, and its dup runs on DVE.
            def chunks(g):
                if g == 0:
                    return [(0, 16), (16, 16), (32, 32)]
                if g == N_GROUPS - 1:
                    # four z16 chunks: steadier store feed at the end and a
                    # small final store, minimizing the exposed HBM drain
                    return [(0, 16), (16, 16), (32, 16), (48, 16)]
                return [(0, 32), (32, 32)]
            t8s = {0: emit_terms(0)}
            stages = {}
            stages[0] = emit_pair(0, t8s[0])
            emit_quad(0, stages[0], 1)
            # group-0 dups on ScalarE, emitted BEFORE terms(1) so they are
            # not head-of-line blocked behind terms(1)'s wait for the g1
            # gather DMAs; they only wait on quadH(0). This frees ~1.2us of
            # DVE head time vs running them as DVE tensor_copies.
            emit_dup(0, stages[0], 0, scalar=True)
            emit_dup(0, stages[0], 32, scalar=True)
            emit_quad(0, stages[0], 0)
            t8s[1] = emit_terms(1)
            for g in range(N_GROUPS):
                cl = chunks(g)
                last = len(cl) - 1
                for ci, (c0, cn) in enumerate(cl):
                    emit_comb(g, stages[g], c0, cn, ci)
                    if g + 1 < N_GROUPS:
                        if ci == 0:
                            stages[g + 1] = emit_pair(g + 1, t8s[g + 1])
                            emit_quad(g + 1, stages[g + 1], 1)
                            emit_dup(g + 1, stages[g + 1], 0)
                            emit_dup(g + 1, stages[g + 1], 32)
                        elif ci == last:
                            emit_quad(g + 1, stages[g + 1], 0)
                            if g + 2 < N_GROUPS:
                                t8s[g + 2] = emit_terms(g + 2)

    if hoist:
        _hoist_extra_waits(nc)
    return nc


_NC_CACHE = None


def _get_program():
    global _NC_CACHE
    if _NC_CACHE is None:
        _NC_CACHE = _build_program()
    return _NC_CACHE


def kernel(occupancy: np.ndarray) -> np.ndarray:
    occupancy = np.asarray(occupancy, dtype=np.float32)
    assert occupancy.shape == (65, 65, 65)
    nc = _get_program()
    in_maps = [
        {"occ": np.ascontiguousarray(occupancy[8 * k:8 * k + 9])}
        for k in range(N_CORES)
    ]
    res = run_bass_kernel_spmd(nc, in_maps, core_ids=list(range(N_CORES)))
    return np.concatenate(
        [np.asarray(res.results[k]["topo"]).astype(np.float32)
         for k in range(N_CORES)], axis=0)

